# revision 3
# baseline (speedup 1.0000x reference)
"""Chamfer one-direction nearest-neighbor distance on 8 Trainium2 NeuronCores.

For each of 65536 query points (pc0) find min squared distance to 65536
points (pc1), then return mean over queries with min-dist <= 2.0.

Strategy:
  - Shard pc0 across the 8 cores (8192 queries each), replicate pc1.
  - Distance matrix tile = single K=20 fp16 matmul via the augmentation
      d = |a|^2 + |b|^2 - 2 a.b  ->  A' . B'
    with A' = [x,y,z,|a|^2,1], B' = [-2x,-2y,-2z,1,|b|^2], each split into
    fp16 hi/lo pairs (products of fp16 pairs are exact in fp32, so the
    distance matrix is fp32-accurate).
  - Per 128-query block, stream candidates through PSUM in [128,1024]
    fp32 tiles; ScalarE evacuates every other tile to SBUF and VectorE
    consumes two tiles per instruction with a running-min
    tensor_tensor_scan (state = min(psum[t], state, sbuf[t])).
  - Masked sum + count per partition on-device; final scalar on host.
"""

import os

os.environ.setdefault("NEURON_RT_RESET_CORES", "1")

import numpy as np

N_CORES = 8

# lhsT rows: [A1, A1, A2, A2] ; rhs rows: [B1, B2, B1, B2]  (5 feats each)
FMAP_A = [0, 1, 2, 3, 4, 0, 1, 2, 3, 4, 5, 6, 7, 8, 9, 5, 6, 7, 8, 9]
FMAP_B = [0, 1, 2, 3, 4, 5, 6, 7, 8, 9, 0, 1, 2, 3, 4, 5, 6, 7, 8, 9]

_STATE = {}


def build_nc(nq=8192, np_total=65536, bases=(0, 32, 64, 96), name_suffix=""):
    """Build the per-core Bass program.

    nq: queries per core; np_total: candidate points (replicated).
    bases: partition bases for the 4 point-quarters.
    """
    import concourse.bacc as bacc
    import concourse.tile as tile
    from concourse import mybir

    f32, f16 = mybir.dt.float32, mybir.dt.float16
    AX, OP = mybir.AxisListType, mybir.AluOpType
    BIG = 1.0e30

    quarter = np_total // 4
    jsteps = quarter // 512
    iblocks = nq // 128
    assert nq % 128 == 0 and np_total % (4 * 512) == 0

    nc = bacc.Bacc("TRN2", target_bir_lowering=False)
    pc0s = nc.dram_tensor("pc0s", [nq, 3], f32, kind="ExternalInput")
    pc1 = nc.dram_tensor("pc1", [np_total, 3], f32, kind="ExternalInput")
    out = nc.dram_tensor("out", [128, 2], f32, kind="ExternalOutput")
    # feature staging in DRAM, 16 fp16 slots per point: [hi(5), lo(5), pad(6)]
    sa = nc.dram_tensor("scratch_a", [nq, 16], f16)
    sb = nc.dram_tensor("scratch_b", [np_total, 16], f16)

    with tile.TileContext(nc) as tc:
        with tc.tile_pool(name="mm", bufs=1) as mmp, \
             tc.tile_pool(name="keep", bufs=1) as keep:

            # ---------------- feature build (point-major, 128-way parallel) ----
            def build_feats(src_ap, n_per_part, scratch, a_side, prep):
                raw = prep.tile([128, n_per_part, 3], f32, tag="raw")
                nc.sync.dma_start(out=raw, in_=src_ap.rearrange("(p n) c -> p n c", p=128))
                sq = prep.tile([128, n_per_part, 3], f32, tag="sq")
                nc.vector.tensor_mul(sq, raw, raw)
                n2 = prep.tile([128, n_per_part], f32, tag="n2")
                nc.vector.tensor_reduce(out=n2, in_=sq, axis=AX.X, op=OP.add)
                aug = prep.tile([128, n_per_part, 5], f32, tag="aug")
                if a_side:
                    nc.vector.tensor_copy(aug[:, :, 0:3], raw)
                    nc.vector.tensor_copy(aug[:, :, 3:4], n2[:, :, None])
                    nc.vector.memset(aug[:, :, 4:5], 1.0)
                else:
                    nc.scalar.mul(aug[:, :, 0:3], raw, -2.0)
                    nc.vector.memset(aug[:, :, 3:4], 1.0)
                    nc.vector.tensor_copy(aug[:, :, 4:5], n2[:, :, None])
                feats = prep.tile([128, n_per_part, 16], f16, tag="feats")
                nc.vector.tensor_copy(feats[:, :, 0:5], aug)
                res = prep.tile([128, n_per_part, 5], f32, tag="res")
                nc.vector.tensor_sub(res, aug, feats[:, :, 0:5])
                nc.vector.tensor_copy(feats[:, :, 5:10], res)
                nc.vector.memset(feats[:, :, 10:16], 0.0)
                nc.sync.dma_start(
                    out=scratch[:, :].rearrange("(p n) c -> p n c", p=128),
                    in_=feats,
                )

            with tc.tile_pool(name="prep", bufs=1) as prep:
                build_feats(pc1[:, :], np_total // 128, sb, False, prep)
                build_feats(pc0s[:, :], nq // 128, sa, True, prep)

            # ---------------- transposed operands via strided DMA readback ----
            rhs = mmp.tile([128, quarter], f16)
            lhsT = mmp.tile([128, nq], f16)
            for qi, base in enumerate(bases):
                for r in range(20):
                    nc.sync.dma_start(
                        out=rhs[base + r : base + r + 1, :],
                        in_=sb[qi * quarter : (qi + 1) * quarter, FMAP_B[r]][None, :],
                    )
                    nc.sync.dma_start(
                        out=lhsT[base + r : base + r + 1, :],
                        in_=sa[:, FMAP_A[r]][None, :],
                    )

            # ---------------- main loop: matmul + running-min scan ------------
            mins = keep.tile([128, iblocks], f32)
            with tc.tile_pool(name="psum", bufs=4, space="PSUM") as pp, \
                 tc.tile_pool(name="work", bufs=3) as wk, \
                 tc.tile_pool(name="scan", bufs=3) as scn:
                for i in range(iblocks):
                    isl = slice(i * 128, (i + 1) * 128)
                    prev = None
                    for t in range(jsteps):
                        jsl = slice(t * 512, (t + 1) * 512)
                        psA = pp.tile([128, 1024], f32, tag="ps")
                        psB = pp.tile([128, 1024], f32, tag="ps")
                        for ps, h, qi in ((psA, 0, 0), (psA, 1, 1), (psB, 0, 2), (psB, 1, 3)):
                            base = bases[qi]
                            nc.tensor.matmul(
                                ps[:, h * 512 : (h + 1) * 512],
                                lhsT[base : base + 20, isl],
                                rhs[base : base + 20, jsl],
                                start=True, stop=True,
                                tile_position=(base, 0),
                            )
                        sbB = wk.tile([128, 1024], f32, tag="actsb")
                        nc.scalar.copy(sbB, psB)
                        tr = scn.tile([128, 1024], f32, tag="tr")
                        init = BIG if prev is None else prev[:, 1023:1024]
                        nc.vector.tensor_tensor_scan(
                            tr, psA, sbB, init, op0=OP.min, op1=OP.min
                        )
                        prev = tr
                    nc.vector.tensor_copy(mins[:, i : i + 1], prev[:, 1023:1024])

            # ---------------- masked sum + count ------------------------------
            mask = keep.tile([128, iblocks], f32)
            nc.vector.tensor_scalar(mask, mins, 2.0, None, op0=OP.is_le)
            masked = keep.tile([128, iblocks], f32)
            nc.vector.tensor_mul(masked, mins, mask)
            acc = keep.tile([128, 2], f32)
            nc.vector.tensor_reduce(out=acc[:, 0:1], in_=masked, axis=AX.X, op=OP.add)
            nc.vector.tensor_reduce(out=acc[:, 1:2], in_=mask, axis=AX.X, op=OP.add)
            nc.sync.dma_start(out=out[:, :], in_=acc)

    nc.finalize()
    return nc


def _get_nc():
    if "nc" not in _STATE:
        _STATE["nc"] = build_nc()
    return _STATE["nc"]


def kernel(pc0, pc1):
    from concourse.bass_utils import run_bass_kernel_spmd

    pc0 = np.ascontiguousarray(np.asarray(pc0, dtype=np.float32))
    pc1 = np.ascontiguousarray(np.asarray(pc1, dtype=np.float32))
    nq = pc0.shape[0] // N_CORES
    nc = _get_nc()
    in_maps = [
        {"pc0s": pc0[c * nq : (c + 1) * nq], "pc1": pc1} for c in range(N_CORES)
    ]
    res = run_bass_kernel_spmd(nc, in_maps, core_ids=list(range(N_CORES)))
    s = 0.0
    cnt = 0.0
    for c in range(N_CORES):
        o = res.results[c]["out"]
        s += float(o[:, 0].sum(dtype=np.float64))
        cnt += float(o[:, 1].sum(dtype=np.float64))
    return np.array(s / cnt, dtype=np.float32)
